# revision 1
# baseline (speedup 1.0000x reference)
"""Trainium2 Bass kernel for a single-head causal attention block.

Reference computation (per batch b):
    q = x @ Wq + bq ; k = x @ Wk + bk ; v = x @ Wv + bv          [T, H]
    wei = softmax(causal(q @ k.T * C**-0.5))                      [T, T]
    out = wei @ v                                                 [T, H]

Sharding over 8 NeuronCores: core = 2*b + half. Each core handles one
batch and half of the T query rows. Row blocks (128 rows each) are
assigned to halves in a balanced zig-zag pattern (0,3,4,7,... vs
1,2,5,6,...) so both halves see the same causal workload profile: the
j-th slot of either half processes a key range of exactly (2j+2)*128
columns, which keeps the on-device program identical across cores (pure
SPMD) — the gap between the slot's key range and the row block's true
causal boundary is handled by a small additive mask supplied as input
data. The host permutes the query rows per half so slot j's rows are
contiguous, and scatters the output rows back.

On-device plan (per core; x and the weights arrive host-pre-transposed /
pre-shuffled so every DMA is descriptor-friendly):
  phase A: project kT (f32r matmuls out of a resident xT), project v
           (bias folded in) into a DRAM scratch in bf16; T is processed
           in halves so xT + kT + qT + weight strips fit SBUF. The qT
           projection is a third pass over the same pools (no barrier).
  phase C: per slot: wei = qT.T @ kT (bf16), additive mask, exp on ACT
           with fused row-sum (no max-subtraction: logits are O(few)
           after the 1/32 scale), transpose P on PE, out = P.T @ v with
           late normalization fused into the PSUM->SBUF copy.
"""

import math

import numpy as np

P = 128
B, T, C, H = 4, 2048, 1024, 2048
HS = 256  # head columns per weight strip
NEG = -1.0e10


def half_blocks(nb: int) -> tuple[list[int], list[int]]:
    h0, h1 = [], []
    for g in range(nb // 4):
        h0 += [4 * g, 4 * g + 3]
        h1 += [4 * g + 1, 4 * g + 2]
    return h0, h1


def make_masks(t: int) -> np.ndarray:
    """masks[half][slot] : [P, 2P] additive mask for the last 2 chunks of
    the slot's key range."""
    nb = t // P
    masks = np.zeros((2, nb // 2, P, 2 * P), dtype=np.float32)
    for half, blocks in enumerate(half_blocks(nb)):
        for j, pb in enumerate(sorted(blocks)):
            s_end = (2 * j + 2) * P
            col = s_end - 2 * P + np.arange(2 * P)[None, :]
            trow = pb * P + np.arange(P)[:, None]
            masks[half, j] = np.where(col <= trow, 0.0, NEG)
    return masks


def build_nc(t: int = T, c: int = C, h: int = H, reps: int = 1, phases: str = "ABC"):
    import concourse.bass as bass
    import concourse.mybir as mybir
    import concourse.tile as tile
    from concourse import bacc
    from concourse.masks import make_identity

    f32 = mybir.dt.float32
    f32r = mybir.dt.float32r
    bf16 = mybir.dt.bfloat16

    nb = t // P          # key/row blocks per batch
    ns = nb // 2         # slots (row blocks per core)
    ck = c // P          # contraction chunks
    hk = h // P          # head chunks
    tq = t // 2          # query rows per core
    hs = HS              # head columns per weight strip
    ts = min(512, t)     # t columns per projection matmul
    scale = float(c) ** -0.5

    nc = bacc.Bacc("TRN2", target_bir_lowering=False, debug=False, num_devices=8)

    xkvT = nc.dram_tensor("xkvT", [c, t], f32, kind="ExternalInput").ap()
    xqT = nc.dram_tensor("xqT", [c, tq], f32, kind="ExternalInput").ap()
    wq = nc.dram_tensor("wq", [h // HS, P, c // P, HS], f32,
                        kind="ExternalInput").ap()
    wk = nc.dram_tensor("wk", [h // HS, P, c // P, HS], f32,
                        kind="ExternalInput").ap()
    wv = nc.dram_tensor("wv", [h // HS, P, c // P, HS], f32,
                        kind="ExternalInput").ap()
    bq = nc.dram_tensor("bq", [h], f32, kind="ExternalInput").ap()
    bk = nc.dram_tensor("bk", [h], f32, kind="ExternalInput").ap()
    bv = nc.dram_tensor("bv", [h], f32, kind="ExternalInput").ap()
    mask = nc.dram_tensor("mask", [ns, P, 2 * P], f32, kind="ExternalInput").ap()
    out = nc.dram_tensor("out", [tq, h], f32, kind="ExternalOutput").ap()

    with tile.TileContext(nc) as tc:
        with (
            tc.tile_pool(name="singles", bufs=1) as singles,
            tc.tile_pool(name="ktp", bufs=1) as ktp,
            tc.tile_pool(name="qtp", bufs=1) as qtp,
            tc.tile_pool(name="vdram", bufs=1, space="DRAM") as vdram,
        ):
            ident16 = singles.tile([P, P], bf16)
            make_identity(nc, ident16)
            bq_t = singles.tile([P, hk], f32)
            nc.sync.dma_start(out=bq_t, in_=bq.rearrange("(k p) -> p k", p=P))
            bk_t = singles.tile([P, hk], f32)
            nc.sync.dma_start(out=bk_t, in_=bk.rearrange("(k p) -> p k", p=P))

            kt = ktp.tile([P, hk, t], bf16)
            qt = qtp.tile([P, hk, tq], bf16)
            vs = vdram.tile([nb, P, h], bf16)

            # ---- phase A: xT, kT, v (T processed in halves to fit SBUF) ----
            tha = max(1, t // 1024)   # t-half passes
            thl = t // tha            # rows per pass
            rep_ctx = range(reps)
            for rep in rep_ctx:
              if "A" in phases:
                with (
                  tc.tile_pool(name="bvp", bufs=1) as bvp,
                  tc.tile_pool(name="xtp", bufs=1) as xtp,
                  tc.tile_pool(name="wkp", bufs=2) as wkp,
                  tc.tile_pool(name="wvp", bufs=2) as wvp,
                  tc.tile_pool(name="vstg", bufs=2) as vstg,
                  tc.tile_pool(name="pr_ps", bufs=4, space="PSUM") as pr_ps,
              ):
                  bv_t = bvp.tile([P, h], f32)
                  bv_bcast = bass.AP(tensor=bv.tensor, offset=bv.offset,
                                     ap=[[0, P], [1, h]])
                  nc.sync.dma_start(out=bv_t, in_=bv_bcast)
                  for th in range(tha):
                      xt = xtp.tile([P, ck, thl], f32r, name=f"xt{th}", tag="xt")
                      nc.sync.dma_start(
                          out=xt,
                          in_=xkvT[:, th * thl:(th + 1) * thl]
                          .rearrange("(k p) t -> p k t", p=P).bitcast(f32r))

                      for hsi in range(h // hs):
                          wk_t = wkp.tile([P, ck, hs], f32r, name=f"wk{th}_{hsi}",
                                          tag="wk")
                          nc.sync.dma_start(out=wk_t, in_=wk[hsi].bitcast(f32r))
                          wv_t = wvp.tile([P, ck, hs], f32r, name=f"wv{th}_{hsi}",
                                          tag="wv")
                          nc.sync.dma_start(out=wv_t, in_=wv[hsi].bitcast(f32r))
                          # kT rows for this strip
                          for h2 in range(hs // P):
                              hh = hsi * (hs // P) + h2
                              for tt in range(thl // ts):
                                  ps = pr_ps.tile([P, ts], f32)
                                  for cc in range(ck):
                                      nc.tensor.matmul(
                                          ps,
                                          lhsT=wk_t[:, cc, h2 * P:(h2 + 1) * P],
                                          rhs=xt[:, cc, tt * ts:(tt + 1) * ts],
                                          start=(cc == 0), stop=(cc == ck - 1))
                                  nc.scalar.activation(
                                      out=kt[:, hh,
                                             th * thl + tt * ts:
                                             th * thl + (tt + 1) * ts],
                                      in_=ps,
                                      func=mybir.ActivationFunctionType.Identity,
                                      bias=bk_t[:, hh:hh + 1])
                          # v columns for this strip
                          vst = vstg.tile([P, thl // P, hs], bf16,
                                          name=f"vst{th}_{hsi}", tag="vst")
                          for sb in range(thl // P):
                              ps = pr_ps.tile([P, hs], f32, tag="vps")
                              for cc in range(ck):
                                  nc.tensor.matmul(
                                      ps,
                                      lhsT=xt[:, cc, sb * P:(sb + 1) * P],
                                      rhs=wv_t[:, cc, :],
                                      start=(cc == 0), stop=(cc == ck - 1))
                              nc.vector.tensor_add(
                                  out=vst[:, sb, :], in0=ps,
                                  in1=bv_t[:, hsi * hs:(hsi + 1) * hs])
                          nc.sync.dma_start(
                              out=vs[th * (thl // P):(th + 1) * (thl // P), :,
                                     hsi * hs:(hsi + 1) * hs]
                              .rearrange("n p h -> p n h"),
                              in_=vst)

                  # ---- q pass (phase B merged: same pools/tags, no barrier) ----
                  if "B" in phases:
                      tsq = min(512, tq)
                      xtq = xtp.tile([P, ck, tq], f32r, name="xtq", tag="xt")
                      nc.sync.dma_start(
                          out=xtq,
                          in_=xqT.rearrange("(k p) t -> p k t", p=P).bitcast(f32r))
                      for hsi in range(h // hs):
                          wq_t = wkp.tile([P, ck, hs], f32r, name=f"wq{hsi}",
                                          tag="wk")
                          nc.sync.dma_start(out=wq_t, in_=wq[hsi].bitcast(f32r))
                          for h2 in range(hs // P):
                              hh = hsi * (hs // P) + h2
                              for tt in range(tq // tsq):
                                  ps = pr_ps.tile([P, tsq], f32, tag="ps")
                                  for cc in range(ck):
                                      nc.tensor.matmul(
                                          ps,
                                          lhsT=wq_t[:, cc, h2 * P:(h2 + 1) * P],
                                          rhs=xtq[:, cc, tt * tsq:(tt + 1) * tsq],
                                          start=(cc == 0), stop=(cc == ck - 1))
                                  nc.scalar.activation(
                                      out=qt[:, hh, tt * tsq:(tt + 1) * tsq],
                                      in_=ps,
                                      func=mybir.ActivationFunctionType.Identity,
                                      bias=bq_t[:, hh:hh + 1])

              if "C" in phases:
                # ---- phase C: attention ----
                with (
                    tc.tile_pool(name="maskp", bufs=1) as maskp,
                    tc.tile_pool(name="weip", bufs=2) as weip,
                    tc.tile_pool(name="pp", bufs=2) as pp,
                    tc.tile_pool(name="ptp", bufs=2) as ptp,
                    tc.tile_pool(name="vinp", bufs=3) as vinp,
                    tc.tile_pool(name="vcachep", bufs=1) as vcachep,
                    tc.tile_pool(name="outp", bufs=2) as outp,
                    tc.tile_pool(name="stats", bufs=8) as stats,
                    tc.tile_pool(name="wei_ps", bufs=2, space="PSUM") as wei_ps,
                    tc.tile_pool(name="pv_ps", bufs=1, space="PSUM") as pv_ps,
                    tc.tile_pool(name="pt_ps", bufs=2, space="PSUM") as pt_ps,
                ):
                    import concourse.mybir as mb
                    mask_t = maskp.tile([P, ns, 2 * P], f32)
                    nc.sync.dma_start(out=mask_t, in_=mask.rearrange("j p c -> p j c"))
                    nstr = h // 512  # output column strips
                    n_vcache = min(2, ns - 1)  # hot v chunk-pairs kept in SBUF
                    vcache: dict = {}
                    for j in range(ns):
                        scn = 2 * j + 2
                        s_end = scn * P
                        wt = weip.tile([P, t], f32)
                        for ss in range(math.ceil(s_end / 512)):
                            w = min(512, s_end - ss * 512)
                            ps = wei_ps.tile([P, 512], f32)
                            for hh in range(hk):
                                nc.tensor.matmul(
                                    ps[:, :w],
                                    lhsT=qt[:, hh, j * P:(j + 1) * P],
                                    rhs=kt[:, hh, ss * 512:ss * 512 + w],
                                    start=(hh == 0), stop=(hh == hk - 1))
                            nc.vector.tensor_copy(out=wt[:, ss * 512:ss * 512 + w],
                                                  in_=ps[:, :w])
                        nc.vector.tensor_add(
                            out=wt[:, s_end - 2 * P:s_end],
                            in0=wt[:, s_end - 2 * P:s_end], in1=mask_t[:, j, :])
                        # no max-subtraction: logits are O(few) after the 1/32
                        # scale, masked entries underflow exp -> 0.
                        pt_t = pp.tile([P, t], bf16)
                        rsum = stats.tile([P, 1], f32)
                        nc.scalar.activation(
                            out=pt_t[:, :s_end], in_=wt[:, :s_end],
                            func=mb.ActivationFunctionType.Exp,
                            scale=scale, accum_out=rsum)
                        rinv = stats.tile([P, 1], f32)
                        nc.vector.reciprocal(rinv, rsum)
                        ptt = ptp.tile([P, nb, P], bf16)
                        for sc in range(scn):
                            pps = pt_ps.tile([P, P], bf16)
                            nc.tensor.transpose(pps, pt_t[:, sc * P:(sc + 1) * P], ident16)
                            nc.scalar.copy(out=ptt[:, sc, :], in_=pps)
                        pv = [pv_ps.tile([P, 512], f32, tag=f"pv{n}", name=f"pv{n}_{j}")
                              for n in range(nstr)]
                        for scp in range(scn // 2):
                            if scp < n_vcache:
                                # hot chunk pairs: loaded once (slot == scp),
                                # served from SBUF for every later slot
                                if j == scp:
                                    vc = vcachep.tile([P, 2, h], bf16,
                                                      name=f"vc{scp}",
                                                      tag=f"vc{scp}")
                                    nc.gpsimd.dma_start(
                                        out=vc,
                                        in_=vs[2 * scp:2 * scp + 2, :, :]
                                        .rearrange("n p h -> p n h"))
                                    vcache[scp] = vc
                                vt = vcache[scp]
                            else:
                                vt = vinp.tile([P, 2, h], bf16)
                                nc.gpsimd.dma_start(
                                    out=vt,
                                    in_=vs[2 * scp:2 * scp + 2, :, :]
                                    .rearrange("n p h -> p n h"))
                            for i in range(2):
                                sc = 2 * scp + i
                                for n in range(nstr):
                                    nc.tensor.matmul(
                                        pv[n], lhsT=ptt[:, sc, :],
                                        rhs=vt[:, i, n * 512:(n + 1) * 512],
                                        start=(sc == 0), stop=(sc == scn - 1))
                        ot = outp.tile([P, h], f32)
                        for n in range(nstr):
                            nc.scalar.activation(
                                out=ot[:, n * 512:(n + 1) * 512], in_=pv[n],
                                func=mb.ActivationFunctionType.Copy, scale=rinv)
                        nc.scalar.dma_start(out=out[j * P:(j + 1) * P, :], in_=ot)

    nc.compile()
    return nc


class Runner:
    """Compiles the per-core program once and runs it on 8 cores via PJRT.

    Mirrors concourse.bass2jax.run_bass_via_pjrt's multi-core path, but
    keeps the jitted executable and device-resident inputs so repeated
    calls don't recompile or re-upload.
    """

    def __init__(self, t: int = T, c: int = C, h: int = H, reps: int = 1,
                 phases: str = "ABC"):
        import jax
        import concourse.mybir as mybir
        from concourse import bass2jax
        from jax.experimental.shard_map import shard_map
        from jax.sharding import Mesh, NamedSharding, PartitionSpec

        bass2jax.install_neuronx_cc_hook()
        self.jax = jax
        nc = build_nc(t, c, h, reps=reps, phases=phases)
        self.nc = nc
        self.n_cores = 8

        partition_name = (nc.partition_id_tensor.name
                          if nc.partition_id_tensor else None)
        in_names, out_names, out_avals, zero_outs = [], [], [], []
        for alloc in nc.m.functions[0].allocations:
            if not isinstance(alloc, mybir.MemoryLocationSet):
                continue
            name = alloc.memorylocations[0].name
            if alloc.kind == "ExternalInput":
                if name != partition_name:
                    in_names.append(name)
            elif alloc.kind == "ExternalOutput":
                shape = tuple(alloc.tensor_shape)
                dtype = mybir.dt.np(alloc.dtype)
                out_names.append(name)
                out_avals.append(jax.core.ShapedArray(shape, dtype))
                zero_outs.append(np.zeros(shape, dtype))
        self.in_names = list(in_names)
        self.out_names = out_names
        self.out_avals = out_avals
        n_params = len(in_names)
        all_in_names = in_names + out_names
        if partition_name is not None:
            all_in_names = all_in_names + [partition_name]

        def _body(*args):
            operands = list(args)
            if partition_name is not None:
                operands.append(bass2jax.partition_id_tensor())
            outs = bass2jax._bass_exec_p.bind(
                *operands,
                out_avals=tuple(out_avals),
                in_names=tuple(all_in_names),
                out_names=tuple(out_names),
                lowering_input_output_aliases=(),
                sim_require_finite=True,
                sim_require_nnan=True,
                nc=nc,
            )
            return tuple(outs)

        devices = jax.devices()[:self.n_cores]
        self.mesh = Mesh(np.asarray(devices), ("core",))
        nspec = (PartitionSpec("core"),) * (n_params + len(out_names))
        self._fn = jax.jit(
            shard_map(_body, mesh=self.mesh, in_specs=nspec,
                      out_specs=(PartitionSpec("core"),) * len(out_names),
                      check_rep=False),
            keep_unused=True)
        self._sharding = NamedSharding(self.mesh, PartitionSpec("core"))
        self._zero_outs = zero_outs

    def stage(self, in_maps: list[dict[str, np.ndarray]]):
        """Upload per-core inputs (list of 8 dicts) to the devices."""
        jax = self.jax
        args = []
        for name in self.in_names:
            cat = np.concatenate([np.asarray(m[name]) for m in in_maps], axis=0)
            args.append(jax.device_put(cat, self._sharding))
        for z in self._zero_outs:
            cat = np.zeros((self.n_cores * z.shape[0], *z.shape[1:]), z.dtype)
            args.append(jax.device_put(cat, self._sharding))
        return args

    def run_staged(self, args):
        return self._fn(*args)

    def __call__(self, in_maps: list[dict[str, np.ndarray]]):
        out_arrs = self.run_staged(self.stage(in_maps))
        self.jax.block_until_ready(out_arrs)
        return [
            {name: np.asarray(out_arrs[i]).reshape(
                self.n_cores, *self.out_avals[i].shape)[cid]
             for i, name in enumerate(self.out_names)}
            for cid in range(self.n_cores)
        ]


_runner_cache: dict = {}


def get_runner(t: int = T, c: int = C, h: int = H, reps: int = 1) -> Runner:
    key = (t, c, h, reps)
    if key not in _runner_cache:
        _runner_cache[key] = Runner(t, c, h, reps)
    return _runner_cache[key]


def _shuffle_w(W, c, h):
    """[c, h] -> [h//HS, P, c//P, HS] so each weight-strip DMA reads one
    contiguous block with 8KB-per-partition descriptor lines."""
    W = np.asarray(W, np.float32).reshape(c // P, P, h // HS, HS)
    return np.ascontiguousarray(W.transpose(2, 1, 0, 3))


def make_in_maps(x, Wq, bq, Wk, bk, Wv, bv):
    """Build the 8 per-core input dicts from full inputs."""
    t = x.shape[1]
    nb = t // P
    blocks = half_blocks(nb)
    masks = make_masks(t)
    x = np.ascontiguousarray(np.asarray(x, dtype=np.float32))
    c, h = x.shape[2], np.asarray(Wq).shape[1]
    wq_s, wk_s, wv_s = (_shuffle_w(W, c, h) for W in (Wq, Wk, Wv))
    in_maps = []
    for core in range(8):
        b, half = divmod(core, 2)
        rows = np.concatenate(
            [np.arange(pb * P, (pb + 1) * P) for pb in sorted(blocks[half])])
        in_maps.append({
            "xkvT": np.ascontiguousarray(x[b].T),
            "xqT": np.ascontiguousarray(x[b][rows].T),
            "wq": wq_s, "wk": wk_s, "wv": wv_s,
            "bq": np.asarray(bq, np.float32), "bk": np.asarray(bk, np.float32),
            "bv": np.asarray(bv, np.float32),
            "mask": masks[half],
        })
    return in_maps


def assemble(results, t, h):
    """Scatter per-core [t/2, h] outputs back to [B, t, h]."""
    nb = t // P
    blocks = half_blocks(nb)
    out = np.empty((B, t, h), dtype=np.float32)
    for core in range(8):
        b, half = divmod(core, 2)
        rows = np.concatenate(
            [np.arange(pb * P, (pb + 1) * P) for pb in sorted(blocks[half])])
        out[b][rows] = results[core]["out"]
    return out


def kernel(x, Wq, bq, Wk, bk, Wv, bv):
    t, c, h = x.shape[1], x.shape[2], Wq.shape[1]
    runner = get_runner(t, c, h)
    results = runner(make_in_maps(x, Wq, bq, Wk, bk, Wv, bv))
    return assemble(results, t, h)



# revision 2
# speedup vs baseline: 1.0258x; 1.0258x over previous
"""Trainium2 Bass kernel for a single-head causal attention block (v3).

Sharding over 8 NeuronCores: core = 2*b + half. Each core handles one
batch and half of the T query rows (zig-zag row-block assignment,
additive mask for the causal boundary — as v1).

v3 vs v1:
  * everything staged in bf16 host-side (halves DMA, enables FWL);
  * phase A is a single pass over T (xT fits SBUF in bf16);
  * the V projection is split across the batch pair by head-halves:
    each core projects v for 1024 of the 2048 head columns, and the
    halves are exchanged with ONE pair AllGather per rep (replica
    groups [[0,1],[2,3],[4,5],[6,7]]). Collective input/output live in
    DRAM bounce buffers; the Tile framework tracks the collective's
    data dependencies, so no manual synchronization is needed.
    Attention streams v chunk-pairs from the AllGather output (rank 0's
    shard = head cols 0:1024, rank 1's = 1024:2048 — physical order,
    so the output needs no column unpermute).
  * kT and q are projected in full on each core (kT duplication stays:
    exchanging it would put the AllGather on the critical path).
"""

import math

import numpy as np

P = 128
B, T, C, H = 4, 2048, 1024, 2048
NEG = -1.0e10


def half_blocks(nb: int) -> tuple[list[int], list[int]]:
    h0, h1 = [], []
    for g in range(nb // 4):
        h0 += [4 * g, 4 * g + 3]
        h1 += [4 * g + 1, 4 * g + 2]
    return h0, h1


def make_masks(t: int) -> np.ndarray:
    nb = t // P
    masks = np.zeros((2, nb // 2, P, 2 * P), dtype=np.float32)
    for half, blocks in enumerate(half_blocks(nb)):
        for j, pb in enumerate(sorted(blocks)):
            s_end = (2 * j + 2) * P
            col = s_end - 2 * P + np.arange(2 * P)[None, :]
            trow = pb * P + np.arange(P)[:, None]
            masks[half, j] = np.where(col <= trow, 0.0, NEG)
    return masks


def build_nc(t: int = T, c: int = C, h: int = H, reps: int = 1):
    import concourse.bass as bass
    import concourse.mybir as mybir
    import concourse.tile as tile
    from concourse import bacc
    from concourse.masks import make_identity

    f32 = mybir.dt.float32
    bf16 = mybir.dt.bfloat16

    nb = t // P
    ns = nb // 2
    ck = c // P
    hk = h // P
    tq = t // 2
    hh2 = h // 2
    scale = float(c) ** -0.5

    nc = bacc.Bacc("TRN2", target_bir_lowering=False, debug=False, num_devices=8)

    xT = nc.dram_tensor("xT", [c, t], bf16, kind="ExternalInput").ap()
    xqT = nc.dram_tensor("xqT", [c, tq], bf16, kind="ExternalInput").ap()
    wk = nc.dram_tensor("wk", [h // 512, P, ck, 512], bf16,
                        kind="ExternalInput").ap()
    wq = nc.dram_tensor("wq", [h // 512, P, ck, 512], bf16,
                        kind="ExternalInput").ap()
    wv = nc.dram_tensor("wv", [hh2 // 512, P, ck, 512], bf16,
                        kind="ExternalInput").ap()
    bq = nc.dram_tensor("bq", [h], f32, kind="ExternalInput").ap()
    bk = nc.dram_tensor("bk", [h], f32, kind="ExternalInput").ap()
    bv = nc.dram_tensor("bv", [hh2], f32, kind="ExternalInput").ap()
    mask = nc.dram_tensor("mask", [ns, P, 2 * P], f32, kind="ExternalInput").ap()
    out = nc.dram_tensor("out", [tq, h], f32, kind="ExternalOutput").ap()

    # v exchange: bounce in [t, hh2]; AllGather out [2, t, hh2]
    vbin = nc.dram_tensor("vbin", [t, hh2], bf16).ap()
    vout = nc.dram_tensor("vout", [2 * t, hh2], bf16).ap()
    groups = [[0, 1], [2, 3], [4, 5], [6, 7]]

    with tile.TileContext(nc) as tc:
        with (
            tc.tile_pool(name="singles", bufs=1) as singles,
            tc.tile_pool(name="ktp", bufs=1) as ktp,
            tc.tile_pool(name="qtp", bufs=1) as qtp,
        ):
            ident16 = singles.tile([P, P], bf16)
            make_identity(nc, ident16)
            bq_t = singles.tile([P, hk], f32)
            nc.sync.dma_start(out=bq_t, in_=bq.rearrange("(k p) -> p k", p=P))
            bk_t = singles.tile([P, hk], f32)
            nc.sync.dma_start(out=bk_t, in_=bk.rearrange("(k p) -> p k", p=P))

            kt = ktp.tile([P, hk, t], bf16)
            qt = qtp.tile([P, hk, tq], bf16)

            for rep in range(reps):
              with (
                  tc.tile_pool(name="bvp", bufs=1) as bvp,
                  tc.tile_pool(name="xtp", bufs=1) as xtp,
                  tc.tile_pool(name="wkp", bufs=2) as wkp,
                  tc.tile_pool(name="wvp", bufs=2) as wvp,
                  tc.tile_pool(name="vstg", bufs=2) as vstg,
                  tc.tile_pool(name="pr_ps", bufs=4, space="PSUM") as pr_ps,
              ):
                bv_t = bvp.tile([P, hh2], f32)
                bv_bcast = bass.AP(tensor=bv.tensor, offset=bv.offset,
                                   ap=[[0, P], [1, hh2]])
                nc.sync.dma_start(out=bv_t, in_=bv_bcast)
                xt = xtp.tile([P, ck, t], bf16, name=f"xt{rep}", tag="xt")
                nc.sync.dma_start(
                    out=xt, in_=xT.rearrange("(k p) t -> p k t", p=P))

                # ---- v (my head-half) first, so the AllGather hides ----
                vb = vbin.rearrange("(n p) c -> p n c", p=P)
                for hsi in range(hh2 // 512):
                    wv_t = wvp.tile([P, ck, 512], bf16, name=f"wv{rep}_{hsi}",
                                    tag="wv")
                    nc.sync.dma_start(out=wv_t, in_=wv[hsi])
                    for tg in range(2):
                        vst = vstg.tile([P, nb // 2, 512], bf16,
                                        name=f"vst{rep}_{hsi}_{tg}", tag="vst")
                        for sbl in range(nb // 2):
                            sbi = tg * (nb // 2) + sbl
                            ps = pr_ps.tile([P, 512], f32, tag="vps")
                            for cci in range(ck):
                                nc.tensor.matmul(
                                    ps,
                                    lhsT=xt[:, cci, sbi * P:(sbi + 1) * P],
                                    rhs=wv_t[:, cci, :],
                                    start=(cci == 0), stop=(cci == ck - 1))
                            nc.vector.tensor_add(
                                out=vst[:, sbl, :], in0=ps,
                                in1=bv_t[:, hsi * 512:(hsi + 1) * 512])
                        nc.sync.dma_start(
                            out=vb[:, tg * (nb // 2):(tg + 1) * (nb // 2),
                                   hsi * 512:(hsi + 1) * 512],
                            in_=vst)
                nc.gpsimd.collective_compute(
                    "AllGather", mybir.AluOpType.bypass,
                    replica_groups=groups,
                    ins=[vbin], outs=[vout])

                # ---- kT (full head dim) ----
                for hsi in range(h // 512):
                    wk_t = wkp.tile([P, ck, 512], bf16, name=f"wk{rep}_{hsi}",
                                    tag="wk")
                    nc.sync.dma_start(out=wk_t, in_=wk[hsi])
                    for h2 in range(4):
                        hh = hsi * 4 + h2
                        for tt in range(t // 512):
                            ps = pr_ps.tile([P, 512], f32)
                            for cci in range(ck):
                                nc.tensor.matmul(
                                    ps,
                                    lhsT=wk_t[:, cci, h2 * P:(h2 + 1) * P],
                                    rhs=xt[:, cci, tt * 512:(tt + 1) * 512],
                                    start=(cci == 0), stop=(cci == ck - 1))
                            nc.scalar.activation(
                                out=kt[:, hh, tt * 512:(tt + 1) * 512],
                                in_=ps,
                                func=mybir.ActivationFunctionType.Identity,
                                bias=bk_t[:, hh:hh + 1])

                # ---- q ----
                xtq = xtp.tile([P, ck, tq], bf16, name=f"xtq{rep}", tag="xt")
                nc.sync.dma_start(
                    out=xtq, in_=xqT.rearrange("(k p) t -> p k t", p=P))
                for hsi in range(h // 512):
                    wq_t = wkp.tile([P, ck, 512], bf16, name=f"wq{rep}_{hsi}",
                                    tag="wk")
                    nc.sync.dma_start(out=wq_t, in_=wq[hsi])
                    for h2 in range(4):
                        hh = hsi * 4 + h2
                        for tt in range(tq // 512):
                            ps = pr_ps.tile([P, 512], f32, tag="ps")
                            for cci in range(ck):
                                nc.tensor.matmul(
                                    ps,
                                    lhsT=wq_t[:, cci, h2 * P:(h2 + 1) * P],
                                    rhs=xtq[:, cci, tt * 512:(tt + 1) * 512],
                                    start=(cci == 0), stop=(cci == ck - 1))
                            nc.scalar.activation(
                                out=qt[:, hh, tt * 512:(tt + 1) * 512],
                                in_=ps,
                                func=mybir.ActivationFunctionType.Identity,
                                bias=bq_t[:, hh:hh + 1])

              # ---- phase C: attention ----
              with (
                  tc.tile_pool(name="maskp", bufs=1) as maskp,
                  tc.tile_pool(name="weip", bufs=2) as weip,
                  tc.tile_pool(name="pp", bufs=2) as pp,
                  tc.tile_pool(name="ptp", bufs=2) as ptp,
                  tc.tile_pool(name="vinp", bufs=3) as vinp,
                  tc.tile_pool(name="vcachep", bufs=1) as vcachep,
                  tc.tile_pool(name="outp", bufs=2) as outp,
                  tc.tile_pool(name="stats", bufs=8) as stats,
                  tc.tile_pool(name="wei_ps", bufs=2, space="PSUM") as wei_ps,
                  tc.tile_pool(name="pv_ps", bufs=1, space="PSUM") as pv_ps,
                  tc.tile_pool(name="pt_ps", bufs=2, space="PSUM") as pt_ps,
              ):
                mask_t = maskp.tile([P, ns, 2 * P], f32)
                nc.sync.dma_start(out=mask_t, in_=mask.rearrange("j p c -> p j c"))
                nstr = h // 512
                n_vcache = 2
                vcache: dict = {}
                # vout halves as [P, nb, hh2] views
                vlo = vout[0:t].rearrange("(n p) c -> p n c", p=P)
                vhi = vout[t:2 * t].rearrange("(n p) c -> p n c", p=P)

                def load_v(vt, scp):
                    nc.gpsimd.dma_start(
                        out=vt[:, :, 0:hh2],
                        in_=vlo[:, 2 * scp:2 * scp + 2, :])
                    nc.gpsimd.dma_start(
                        out=vt[:, :, hh2:h],
                        in_=vhi[:, 2 * scp:2 * scp + 2, :])

                for j in range(ns):
                    scn = 2 * j + 2
                    s_end = scn * P
                    wt = weip.tile([P, t], f32)
                    for ss in range(math.ceil(s_end / 512)):
                        w = min(512, s_end - ss * 512)
                        ps = wei_ps.tile([P, 512], f32)
                        for hh in range(hk):
                            nc.tensor.matmul(
                                ps[:, :w],
                                lhsT=qt[:, hh, j * P:(j + 1) * P],
                                rhs=kt[:, hh, ss * 512:ss * 512 + w],
                                start=(hh == 0), stop=(hh == hk - 1))
                        nc.vector.tensor_copy(out=wt[:, ss * 512:ss * 512 + w],
                                              in_=ps[:, :w])
                    nc.vector.tensor_add(
                        out=wt[:, s_end - 2 * P:s_end],
                        in0=wt[:, s_end - 2 * P:s_end], in1=mask_t[:, j, :])
                    pt_t = pp.tile([P, t], bf16)
                    rsum = stats.tile([P, 1], f32)
                    nc.scalar.activation(
                        out=pt_t[:, :s_end], in_=wt[:, :s_end],
                        func=mybir.ActivationFunctionType.Exp,
                        scale=scale, accum_out=rsum)
                    rinv = stats.tile([P, 1], f32)
                    nc.vector.reciprocal(rinv, rsum)
                    ptt = ptp.tile([P, nb, P], bf16)
                    for sc in range(scn):
                        pps = pt_ps.tile([P, P], bf16)
                        nc.tensor.transpose(pps, pt_t[:, sc * P:(sc + 1) * P],
                                            ident16)
                        nc.scalar.copy(out=ptt[:, sc, :], in_=pps)
                    pv = [pv_ps.tile([P, 512], f32, tag=f"pv{n}",
                                     name=f"pv{n}_{rep}_{j}")
                          for n in range(nstr)]
                    for scp in range(scn // 2):
                        if scp < n_vcache:
                            if j == scp:
                                vc = vcachep.tile([P, 2, h], bf16,
                                                  name=f"vc{rep}_{scp}",
                                                  tag=f"vc{scp}")
                                load_v(vc, scp)
                                vcache[scp] = vc
                            vt = vcache[scp]
                        else:
                            vt = vinp.tile([P, 2, h], bf16)
                            load_v(vt, scp)
                        for i in range(2):
                            sc = 2 * scp + i
                            for n in range(nstr):
                                nc.tensor.matmul(
                                    pv[n], lhsT=ptt[:, sc, :],
                                    rhs=vt[:, i, n * 512:(n + 1) * 512],
                                    start=(sc == 0), stop=(sc == scn - 1))
                    ot = outp.tile([P, h], f32)
                    for n in range(nstr):
                        nc.scalar.activation(
                            out=ot[:, n * 512:(n + 1) * 512], in_=pv[n],
                            func=mybir.ActivationFunctionType.Copy, scale=rinv)
                    nc.scalar.dma_start(out=out[j * P:(j + 1) * P, :], in_=ot)

    nc.compile()
    return nc


class Runner:
    """Compiles the per-core program once and runs it on 8 cores via PJRT."""

    def __init__(self, t: int = T, c: int = C, h: int = H, reps: int = 1):
        import jax
        import concourse.mybir as mybir
        from concourse import bass2jax
        from jax.experimental.shard_map import shard_map
        from jax.sharding import Mesh, NamedSharding, PartitionSpec

        bass2jax.install_neuronx_cc_hook()
        self.jax = jax
        nc = build_nc(t, c, h, reps=reps)
        self.nc = nc
        self.n_cores = 8

        partition_name = (nc.partition_id_tensor.name
                          if nc.partition_id_tensor else None)
        in_names, out_names, out_avals, zero_outs = [], [], [], []
        for alloc in nc.m.functions[0].allocations:
            if not isinstance(alloc, mybir.MemoryLocationSet):
                continue
            name = alloc.memorylocations[0].name
            if alloc.kind == "ExternalInput":
                if name != partition_name:
                    in_names.append(name)
            elif alloc.kind == "ExternalOutput":
                shape = tuple(alloc.tensor_shape)
                dtype = mybir.dt.np(alloc.dtype)
                out_names.append(name)
                out_avals.append(jax.core.ShapedArray(shape, dtype))
                zero_outs.append(np.zeros(shape, dtype))
        self.in_names = list(in_names)
        self.out_names = out_names
        self.out_avals = out_avals
        n_params = len(in_names)
        all_in_names = in_names + out_names
        if partition_name is not None:
            all_in_names = all_in_names + [partition_name]

        def _body(*args):
            operands = list(args)
            if partition_name is not None:
                operands.append(bass2jax.partition_id_tensor())
            outs = bass2jax._bass_exec_p.bind(
                *operands,
                out_avals=tuple(out_avals),
                in_names=tuple(all_in_names),
                out_names=tuple(out_names),
                lowering_input_output_aliases=(),
                sim_require_finite=True,
                sim_require_nnan=True,
                nc=nc,
            )
            return tuple(outs)

        devices = jax.devices()[:self.n_cores]
        self.mesh = Mesh(np.asarray(devices), ("core",))
        nspec = (PartitionSpec("core"),) * (n_params + len(out_names))
        self._fn = jax.jit(
            shard_map(_body, mesh=self.mesh, in_specs=nspec,
                      out_specs=(PartitionSpec("core"),) * len(out_names),
                      check_rep=False),
            keep_unused=True)
        self._sharding = NamedSharding(self.mesh, PartitionSpec("core"))
        self._zero_outs = zero_outs

    def stage(self, in_maps: list[dict[str, np.ndarray]]):
        jax = self.jax
        args = []
        for name in self.in_names:
            cat = np.concatenate([np.asarray(m[name]) for m in in_maps], axis=0)
            args.append(jax.device_put(cat, self._sharding))
        for z in self._zero_outs:
            cat = np.zeros((self.n_cores * z.shape[0], *z.shape[1:]), z.dtype)
            args.append(jax.device_put(cat, self._sharding))
        return args

    def run_staged(self, args):
        return self._fn(*args)

    def __call__(self, in_maps: list[dict[str, np.ndarray]]):
        out_arrs = self.run_staged(self.stage(in_maps))
        self.jax.block_until_ready(out_arrs)
        return [
            {name: np.asarray(out_arrs[i]).reshape(
                self.n_cores, *self.out_avals[i].shape)[cid]
             for i, name in enumerate(self.out_names)}
            for cid in range(self.n_cores)
        ]


_runner_cache: dict = {}


def get_runner(t: int = T, c: int = C, h: int = H, reps: int = 1) -> Runner:
    key = (t, c, h, reps)
    if key not in _runner_cache:
        _runner_cache[key] = Runner(t, c, h, reps)
    return _runner_cache[key]


def _shuffle_w(Wcols, c):
    """[c, hcols] f32 -> [hcols//512, P, c//P, 512] bf16 strip layout."""
    import ml_dtypes  # noqa: F401
    hcols = Wcols.shape[1]
    W = np.asarray(Wcols, np.float32).reshape(c // P, P, hcols // 512, 512)
    return np.ascontiguousarray(W.transpose(2, 1, 0, 3)).astype("bfloat16")


def make_in_maps(x, Wq, bq, Wk, bk, Wv, bv):
    import ml_dtypes  # noqa: F401
    t = x.shape[1]
    nb = t // P
    blocks = half_blocks(nb)
    masks = make_masks(t)
    x = np.asarray(x, dtype=np.float32)
    c, h = x.shape[2], np.asarray(Wq).shape[1]
    hh2 = h // 2
    Wq = np.asarray(Wq, np.float32)
    Wk = np.asarray(Wk, np.float32)
    Wv = np.asarray(Wv, np.float32)
    bq_f = np.asarray(bq, np.float32)
    bk_f = np.asarray(bk, np.float32)
    bv_f = np.asarray(bv, np.float32)
    wq_s = _shuffle_w(Wq, c)
    wk_s = _shuffle_w(Wk, c)
    wv_halves = [_shuffle_w(Wv[:, half * hh2:(half + 1) * hh2], c)
                 for half in range(2)]
    in_maps = []
    for core in range(8):
        b, half = divmod(core, 2)
        rows = np.concatenate(
            [np.arange(pb * P, (pb + 1) * P) for pb in sorted(blocks[half])])
        xb = x[b].T.astype("bfloat16")
        in_maps.append({
            "xT": np.ascontiguousarray(xb),
            "xqT": np.ascontiguousarray(x[b][rows].T.astype("bfloat16")),
            "wq": wq_s, "wk": wk_s, "wv": wv_halves[half],
            "bq": bq_f, "bk": bk_f,
            "bv": bv_f[half * hh2:(half + 1) * hh2],
            "mask": masks[half],
        })
    return in_maps


def assemble(results, t, h):
    nb = t // P
    blocks = half_blocks(nb)
    out = np.empty((B, t, h), dtype=np.float32)
    for core in range(8):
        b, half = divmod(core, 2)
        rows = np.concatenate(
            [np.arange(pb * P, (pb + 1) * P) for pb in sorted(blocks[half])])
        out[b][rows] = results[core]["out"]
    return out


def kernel(x, Wq, bq, Wk, bk, Wv, bv):
    t, c, h = x.shape[1], x.shape[2], Wq.shape[1]
    runner = get_runner(t, c, h)
    results = runner(make_in_maps(x, Wq, bq, Wk, bk, Wv, bv))
    return assemble(results, t, h)
